# revision 19
# baseline (speedup 1.0000x reference)
"""BF15 linear layer for Trainium2, 8-core data-parallel.

Reference semantics:
  y = bf16(bf15(x) @ W.T); y = bf16(fp32(y) + bias)

Strategy:
- Shard x over tokens (32768 -> 8 x 4096), replicate W + bias.
- Host-side layout prep (part of the distribution strategy): x shards and W
  are fed pre-transposed so the contraction dim (in_features) lands on SBUF
  partitions with fully contiguous DMA; W is repackaged once on the host
  into the matmul dtype.
- On device: bf15-truncate x (clear the low 17 mantissa bits — exact
  truncation toward zero to 6 explicit mantissa bits), cast to the matmul
  dtype, and run the matmuls with fp32 PSUM accumulation.

Two precision modes:
- "fp16x1": single fp16 pass. bf15(x) (7 sig bits) is exact in fp16;
  products bf15(x) * fp16(W) are exact in fp32; the only deviation from the
  fp32 reference matmul is fp16 quantization of W (~2^-11 relative), giving
  ~1e-4 L2 relative error on the bf16 output - at the same level as the
  accumulation-order noise of an exact implementation.
- "bf16x2": W split on host into W_hi = bf16(W), W_lo = bf16(W - W_hi);
  two bf16 passes accumulate in the same PSUM bank, matching the fp32
  reference to ~2^-18. 2x the PE work of fp16x1.
"""

import numpy as np
import ml_dtypes

MODE = "fp16x1"  # "fp16x1" | "bf16x2"

# Problem shape (hardcoded per contract).
B, S, IN, OUT = 8, 4096, 1024, 4096
N_CORES = 8
M = B * S // N_CORES  # tokens per core = 4096

P = 128
KO = IN // P  # 8 k-subtiles
N_CHUNK = 512
N_CHUNKS = OUT // N_CHUNK  # 8
M_STAGE = 512  # tokens staged per x DMA
M_SUB = 128  # tokens per matmul (output partitions)

_NC = {}
LAST_RESULTS = None


def _build(mode):
    from concourse import bacc
    import concourse.mybir as mybir
    import concourse.tile as tile
    from concourse.bass import ds, ts

    f32 = mybir.dt.float32
    bf16 = mybir.dt.bfloat16
    f16 = mybir.dt.float16
    u32 = mybir.dt.uint32
    mm_dt = f16 if mode == "fp16x1" else bf16
    n_pass = 1 if mode == "fp16x1" else 2

    nc = bacc.Bacc("TRN2", target_bir_lowering=False, debug=False,
                   num_devices=N_CORES)
    xt = nc.dram_tensor("xt", [IN, M], f32, kind="ExternalInput")
    # W inputs already transposed + repackaged on host.
    w_ins = []
    for p in range(n_pass):
        w_ins.append(nc.dram_tensor(f"wt{p}", [IN, OUT], mm_dt,
                                    kind="ExternalInput"))
    bias = nc.dram_tensor("bias", [OUT], f32, kind="ExternalInput")
    y = nc.dram_tensor("y", [M, OUT], bf16, kind="ExternalOutput")

    xr = xt.ap().rearrange("(ko ki) m -> ki ko m", ki=P)  # [128, 8, M]
    wrs = [w.ap().rearrange("(ko ki) n -> ki ko n", ki=P) for w in w_ins]
    yr = y.ap()

    N_WARM = 40

    # --- arrival-order schedule -------------------------------------------
    # During the first ~60us the kernel is DMA-paced: W chunks and x stages
    # stream in while the PE computes.  Emit matmul groups (sub, chunk) in
    # the order their inputs are predicted to arrive so the PE never starves.
    if mode == "fp16x1":
        stage_list = [(0, 128), (128, 128), (256, 256)] + \
            [(512 + 512 * i, 512) for i in range((M - 512) // 512)]
        # predicted ready times (us) per the queue plan below:
        # qSP: x stages with W chunks 3..7 interleaved; qAct: W0 (per-ko),
        # W1, W2, bias, then output stores.
        tx_stage = [16.0, 19.0, 31.0, 42.0, 53.0, 67.0, 80.0, 90.0, 98.5, 108.0]
        tw = [20.2, 26.1, 32.9, 21.6, 33.2, 41.7, 55.5, 69.9]
        sub_stage = []   # sub index -> stage index
        tx_sub = []
        for si, (s0, sz) in enumerate(stage_list):
            for _ in range(sz // M_SUB):
                sub_stage.append(si)
                tx_sub.append(tx_stage[si])
        n_subs = len(tx_sub)
        pairs = [(max(tx_sub[sub], tw[c]), sub, c)
                 for sub in range(n_subs) for c in range(N_CHUNKS)]
        pairs.sort(key=lambda t: (t[0], t[1], t[2]))
        order = [(sub, c) for _, sub, c in pairs]
    else:
        stage_list = [(0, 128), (128, 128), (256, 256)] + \
            [(512 + 512 * i, 512) for i in range((M - 512) // 512)]
        sub_stage = []
        for si, (s0, sz) in enumerate(stage_list):
            for _ in range(sz // M_SUB):
                sub_stage.append(si)
        n_subs = len(sub_stage)
        order = [(sub, c) for sub in range(n_subs) for c in range(N_CHUNKS)]

    sub_m0 = []
    for si, (s0, sz) in enumerate(stage_list):
        for j in range(sz // M_SUB):
            sub_m0.append(s0 + j * M_SUB)

    with tile.TileContext(nc) as tc:
        with (
            tc.tile_pool(name="const", bufs=1) as const,
            tc.tile_pool(name="brow", bufs=1) as brow,
            tc.tile_pool(name="xin", bufs=2) as xin,
            tc.tile_pool(name="xmm", bufs=3) as xmmp,
            tc.tile_pool(name="yout", bufs=8) as yout,
            tc.tile_pool(name="psum", bufs=1, space="PSUM") as psum,
        ):
            # PE warmup: zero matmuls while the first DMAs are in flight.
            # Keeps the HAM clock gate open so real matmuls start at 2.4 GHz.
            wz = const.tile([P, N_CHUNK], mm_dt, tag="warm")
            nc.gpsimd.memset(wz[:], 0.0)
            w_sb = [[None] * N_CHUNKS for _ in range(n_pass)]
            for p in range(n_pass):
                for nci in range(N_CHUNKS):
                    w_sb[p][nci] = const.tile([P, KO, N_CHUNK], mm_dt,
                                              name=f"w{p}_{nci}",
                                              tag=f"w{p}_{nci}")
            # W chunks 6,7 have 35-50us of slack: route them via the GpSimd
            # SWDGE queue to free qSP bandwidth for the x stages.
            for p in range(n_pass):
                for nci in (6, 7):
                    nc.gpsimd.dma_start(w_sb[p][nci][:],
                                        wrs[p][:, :, ts(nci, N_CHUNK)])
            pw = psum.tile([P, N_CHUNK], f32, tag="ps0")
            for _ in range(N_WARM):
                nc.tensor.matmul(pw[:], wz[:, :P], wz[:], start=True, stop=True)

            # W tiles: chunks 0-2 + bias on the ACT HWDGE queue, chunks
            # 3-7 interleaved between x stages on the SP queue (load_stage).
            # W chunks 0-2 on qAct; chunks 0 and 1 split per-ko so the
            # first matmul groups can start as soon as the early slices land.
            for p in range(n_pass):
                for ko in range(KO):
                    nc.scalar.dma_start(w_sb[p][0][:, ko, :],
                                        wrs[p][:, ko, ts(0, N_CHUNK)])
            for p in range(n_pass):
                for nci in (1, 2):
                    nc.scalar.dma_start(w_sb[p][nci][:],
                                        wrs[p][:, :, ts(nci, N_CHUNK)])
            bias_row = brow.tile([1, OUT], f32, tag="brow")
            nc.scalar.dma_start(bias_row[:], bias.ap()[None, :])
            bias_sb = const.tile([P, OUT], f32, tag="bias")
            nc.gpsimd.partition_broadcast(bias_sb[:], bias_row[:])

            resident_x = (mode == "fp16x1")
            xmm_tiles = [None] * len(stage_list)

            def load_stage(si):
                s0, sz = stage_list[si]
                xstage = xin.tile([P, KO, M_STAGE], f32, tag="xstage")
                nc.sync.dma_start(xstage[:, :, :sz], xr[:, :, s0:s0 + sz])
                wq = {1: 3, 2: 4, 3: 5}.get(si)
                if wq is not None:  # W chunks 3..7 interleave on qSP
                    for p in range(n_pass):
                        nc.sync.dma_start(w_sb[p][wq][:],
                                          wrs[p][:, :, ts(wq, N_CHUNK)])
                # bf15: truncate toward zero to 6 explicit mantissa bits
                nc.vector.tensor_scalar(
                    xstage[:, :, :sz].bitcast(u32),
                    xstage[:, :, :sz].bitcast(u32),
                    0xFFFE0000, None, mybir.AluOpType.bitwise_and)
                if resident_x:
                    xmm = const.tile([P, KO, sz], mm_dt, name=f"xmm{si}",
                                     tag=f"xmm{si}")
                else:
                    xmm = xmmp.tile([P, KO, M_STAGE], mm_dt, tag="xmm")
                nc.vector.tensor_copy(xmm[:, :, :sz] if not resident_x else xmm[:],
                                      xstage[:, :, :sz])
                xmm_tiles[si] = xmm

            loaded = [False] * len(stage_list)
            for gi, (sub, nci) in enumerate(order):
                si = sub_stage[sub]
                if not loaded[si]:
                    # keep qSP ahead: issue this and the next stage's load
                    for sj in (si, si + 1):
                        if sj < len(stage_list) and not loaded[sj]:
                            load_stage(sj)
                            loaded[sj] = True
                m0 = sub_m0[sub]
                s0 = stage_list[si][0]
                xmm = xmm_tiles[si]
                ps = psum.tile([P, N_CHUNK], f32, tag=f"ps{gi % 8}",
                               name=f"ps{gi % 8}")
                lhs = xmm[:, :, ds(m0 - s0, M_SUB)]
                n_mm = KO * n_pass
                i_mm = 0
                for ko in range(KO):
                    for p in range(n_pass):
                        nc.tensor.matmul(
                            ps[:], lhs[:, ko, :], w_sb[p][nci][:, ko, :],
                            start=(i_mm == 0), stop=(i_mm == n_mm - 1))
                        i_mm += 1
                ysb = yout.tile([P, N_CHUNK], bf16, tag="ysb")
                # round to bf16 first (matches reference), then +bias
                nc.scalar.copy(ysb[:], ps[:])
                nc.vector.tensor_tensor(
                    ysb[:], ysb[:], bias_sb[:, ts(nci, N_CHUNK)],
                    mybir.AluOpType.add)
                nc.scalar.dma_start(
                    yr[m0:m0 + M_SUB, ts(nci, N_CHUNK)], ysb[:])
    nc.compile()
    return nc


def _get_nc(mode):
    if mode not in _NC:
        _NC[mode] = _build(mode)
    return _NC[mode]


def _prep_w(weight, mode):
    wt = weight.astype(np.float32, copy=False).T  # [IN, OUT]
    if mode == "fp16x1":
        return [np.ascontiguousarray(wt.astype(np.float16))]
    w_hi = wt.astype(ml_dtypes.bfloat16)
    w_lo = (wt - w_hi.astype(np.float32)).astype(ml_dtypes.bfloat16)
    return [np.ascontiguousarray(w_hi), np.ascontiguousarray(w_lo)]


def kernel(x: np.ndarray, weight: np.ndarray, bias: np.ndarray) -> np.ndarray:
    from concourse.bass_utils import run_bass_kernel_spmd

    global LAST_RESULTS
    nc = _get_nc(MODE)

    x2d = np.ascontiguousarray(x, dtype=np.float32).reshape(B * S, IN)
    ws = _prep_w(weight, MODE)
    bias = np.ascontiguousarray(bias, dtype=np.float32)

    in_maps = []
    for c in range(N_CORES):
        shard = x2d[c * M:(c + 1) * M]
        im = {"xt": np.ascontiguousarray(shard.T), "bias": bias}
        for p, w in enumerate(ws):
            im[f"wt{p}"] = w
        in_maps.append(im)

    LAST_RESULTS = run_bass_kernel_spmd(
        nc, in_maps, core_ids=list(range(N_CORES)))
    out = np.concatenate(
        [LAST_RESULTS.results[c]["y"] for c in range(N_CORES)], axis=0)
    return out.reshape(B, S, OUT).astype(ml_dtypes.bfloat16, copy=False)


# revision 20
# speedup vs baseline: 1.0575x; 1.0575x over previous
"""BF15 linear layer for Trainium2, 8-core data-parallel.

Reference semantics:
  y = bf16(bf15(x) @ W.T); y = bf16(fp32(y) + bias)

Strategy:
- Shard x over tokens (32768 -> 8 x 4096), replicate W + bias.
- Host-side layout prep (part of the distribution strategy): x shards and W
  are fed pre-transposed so the contraction dim (in_features) lands on SBUF
  partitions with fully contiguous DMA; W is repackaged once on the host
  into the matmul dtype.
- On device: bf15-truncate x (clear the low 17 mantissa bits — exact
  truncation toward zero to 6 explicit mantissa bits), cast to the matmul
  dtype, and run the matmuls with fp32 PSUM accumulation.

Two precision modes:
- "fp16x1": single fp16 pass. bf15(x) (7 sig bits) is exact in fp16;
  products bf15(x) * fp16(W) are exact in fp32; the only deviation from the
  fp32 reference matmul is fp16 quantization of W (~2^-11 relative), giving
  ~1e-4 L2 relative error on the bf16 output - at the same level as the
  accumulation-order noise of an exact implementation.
- "bf16x2": W split on host into W_hi = bf16(W), W_lo = bf16(W - W_hi);
  two bf16 passes accumulate in the same PSUM bank, matching the fp32
  reference to ~2^-18. 2x the PE work of fp16x1.
"""

import numpy as np
import ml_dtypes

MODE = "fp16x1"  # "fp16x1" | "bf16x2"

# Problem shape (hardcoded per contract).
B, S, IN, OUT = 8, 4096, 1024, 4096
N_CORES = 8
M = B * S // N_CORES  # tokens per core = 4096

P = 128
KO = IN // P  # 8 k-subtiles
N_CHUNK = 512
N_CHUNKS = OUT // N_CHUNK  # 8
M_STAGE = 512  # tokens staged per x DMA
M_SUB = 128  # tokens per matmul (output partitions)

_NC = {}
LAST_RESULTS = None


def _build(mode):
    from concourse import bacc
    import concourse.mybir as mybir
    import concourse.tile as tile
    from concourse.bass import ds, ts

    f32 = mybir.dt.float32
    bf16 = mybir.dt.bfloat16
    f16 = mybir.dt.float16
    u32 = mybir.dt.uint32
    mm_dt = f16 if mode == "fp16x1" else bf16
    n_pass = 1 if mode == "fp16x1" else 2

    nc = bacc.Bacc("TRN2", target_bir_lowering=False, debug=False,
                   num_devices=N_CORES)
    xt = nc.dram_tensor("xt", [IN, M], f32, kind="ExternalInput")
    # W inputs already transposed + repackaged on host.
    w_ins = []
    for p in range(n_pass):
        w_ins.append(nc.dram_tensor(f"wt{p}", [IN, OUT], mm_dt,
                                    kind="ExternalInput"))
    bias = nc.dram_tensor("bias", [OUT], f32, kind="ExternalInput")
    y = nc.dram_tensor("y", [M, OUT], bf16, kind="ExternalOutput")

    xr = xt.ap().rearrange("(ko ki) m -> ki ko m", ki=P)  # [128, 8, M]
    wrs = [w.ap().rearrange("(ko ki) n -> ki ko n", ki=P) for w in w_ins]
    yr = y.ap()

    N_WARM = 40

    # --- arrival-order schedule -------------------------------------------
    # During the first ~60us the kernel is DMA-paced: W chunks and x stages
    # stream in while the PE computes.  Emit matmul groups (sub, chunk) in
    # the order their inputs are predicted to arrive so the PE never starves.
    if mode == "fp16x1":
        stage_list = [(0, 128), (128, 128), (256, 256)] + \
            [(512 + 512 * i, 512) for i in range((M - 512) // 512)]
        # predicted ready times (us) per the queue plan below:
        # qSP: x stages with W chunks 3..7 interleaved; qAct: W0 (per-ko),
        # W1, W2, bias, then output stores.
        tx_stage = [16.0, 19.0, 31.0, 42.0, 53.0, 67.0, 80.0, 90.0, 98.5, 108.0]
        tw = [20.2, 26.1, 32.9, 21.6, 33.2, 41.7, 55.5, 69.9]
        sub_stage = []   # sub index -> stage index
        tx_sub = []
        for si, (s0, sz) in enumerate(stage_list):
            for _ in range(sz // M_SUB):
                sub_stage.append(si)
                tx_sub.append(tx_stage[si])
        n_subs = len(tx_sub)
        pairs = [(max(tx_sub[sub], tw[c]), sub, c)
                 for sub in range(n_subs) for c in range(N_CHUNKS)]
        pairs.sort(key=lambda t: (t[0], t[1], t[2]))
        order = [(sub, c) for _, sub, c in pairs]
    else:
        stage_list = [(0, 128), (128, 128), (256, 256)] + \
            [(512 + 512 * i, 512) for i in range((M - 512) // 512)]
        sub_stage = []
        for si, (s0, sz) in enumerate(stage_list):
            for _ in range(sz // M_SUB):
                sub_stage.append(si)
        n_subs = len(sub_stage)
        order = [(sub, c) for sub in range(n_subs) for c in range(N_CHUNKS)]

    sub_m0 = []
    for si, (s0, sz) in enumerate(stage_list):
        for j in range(sz // M_SUB):
            sub_m0.append(s0 + j * M_SUB)

    with tile.TileContext(nc) as tc:
        with (
            tc.tile_pool(name="const", bufs=1) as const,
            tc.tile_pool(name="brow", bufs=1) as brow,
            tc.tile_pool(name="xin", bufs=2) as xin,
            tc.tile_pool(name="xmm", bufs=3) as xmmp,
            tc.tile_pool(name="yout", bufs=8) as yout,
            tc.tile_pool(name="psum", bufs=1, space="PSUM") as psum,
        ):
            # PE warmup: zero matmuls while the first DMAs are in flight.
            # Keeps the HAM clock gate open so real matmuls start at 2.4 GHz.
            wz = const.tile([P, N_CHUNK], mm_dt, tag="warm")
            nc.gpsimd.memset(wz[:], 0.0)
            pw = psum.tile([P, N_CHUNK], f32, tag="ps0")
            for _ in range(N_WARM):
                nc.tensor.matmul(pw[:], wz[:, :P], wz[:], start=True, stop=True)

            # W tiles: chunks 0-2 + bias on the ACT HWDGE queue, chunks
            # 3-7 interleaved between x stages on the SP queue (load_stage).
            w_sb = [[None] * N_CHUNKS for _ in range(n_pass)]
            for p in range(n_pass):
                for nci in range(N_CHUNKS):
                    w_sb[p][nci] = const.tile([P, KO, N_CHUNK], mm_dt,
                                              name=f"w{p}_{nci}",
                                              tag=f"w{p}_{nci}")
            # W chunks 0-2 on qAct; chunks 0 and 1 split per-ko so the
            # first matmul groups can start as soon as the early slices land.
            for p in range(n_pass):
                for ko in range(KO):
                    nc.scalar.dma_start(w_sb[p][0][:, ko, :],
                                        wrs[p][:, ko, ts(0, N_CHUNK)])
            for p in range(n_pass):
                for nci in (1, 2):
                    nc.scalar.dma_start(w_sb[p][nci][:],
                                        wrs[p][:, :, ts(nci, N_CHUNK)])
            bias_row = brow.tile([1, OUT], f32, tag="brow")
            nc.scalar.dma_start(bias_row[:], bias.ap()[None, :])
            bias_sb = const.tile([P, OUT], f32, tag="bias")
            nc.gpsimd.partition_broadcast(bias_sb[:], bias_row[:])

            resident_x = (mode == "fp16x1")
            xmm_tiles = [None] * len(stage_list)

            def load_stage(si):
                s0, sz = stage_list[si]
                xstage = xin.tile([P, KO, M_STAGE], f32, tag="xstage")
                nc.sync.dma_start(xstage[:, :, :sz], xr[:, :, s0:s0 + sz])
                wq = {1: 3, 2: 4, 3: 5, 4: 6, 5: 7}.get(si)
                if wq is not None:  # W chunks 3..7 interleave on qSP
                    for p in range(n_pass):
                        nc.sync.dma_start(w_sb[p][wq][:],
                                          wrs[p][:, :, ts(wq, N_CHUNK)])
                # bf15: truncate toward zero to 6 explicit mantissa bits
                nc.vector.tensor_scalar(
                    xstage[:, :, :sz].bitcast(u32),
                    xstage[:, :, :sz].bitcast(u32),
                    0xFFFE0000, None, mybir.AluOpType.bitwise_and)
                if resident_x:
                    xmm = const.tile([P, KO, sz], mm_dt, name=f"xmm{si}",
                                     tag=f"xmm{si}")
                else:
                    xmm = xmmp.tile([P, KO, M_STAGE], mm_dt, tag="xmm")
                nc.vector.tensor_copy(xmm[:, :, :sz] if not resident_x else xmm[:],
                                      xstage[:, :, :sz])
                xmm_tiles[si] = xmm

            loaded = [False] * len(stage_list)
            for gi, (sub, nci) in enumerate(order):
                si = sub_stage[sub]
                if not loaded[si]:
                    # keep qSP ahead: issue this and the next stage's load
                    for sj in (si, si + 1):
                        if sj < len(stage_list) and not loaded[sj]:
                            load_stage(sj)
                            loaded[sj] = True
                m0 = sub_m0[sub]
                s0 = stage_list[si][0]
                xmm = xmm_tiles[si]
                ps = psum.tile([P, N_CHUNK], f32, tag=f"ps{gi % 8}",
                               name=f"ps{gi % 8}")
                lhs = xmm[:, :, ds(m0 - s0, M_SUB)]
                n_mm = KO * n_pass
                i_mm = 0
                for ko in range(KO):
                    for p in range(n_pass):
                        nc.tensor.matmul(
                            ps[:], lhs[:, ko, :], w_sb[p][nci][:, ko, :],
                            start=(i_mm == 0), stop=(i_mm == n_mm - 1))
                        i_mm += 1
                ysb = yout.tile([P, N_CHUNK], bf16, tag="ysb")
                # round to bf16 first (matches reference), then +bias
                nc.scalar.copy(ysb[:], ps[:])
                nc.vector.tensor_tensor(
                    ysb[:], ysb[:], bias_sb[:, ts(nci, N_CHUNK)],
                    mybir.AluOpType.add)
                nc.scalar.dma_start(
                    yr[m0:m0 + M_SUB, ts(nci, N_CHUNK)], ysb[:])
    nc.compile()
    return nc


def _get_nc(mode):
    if mode not in _NC:
        _NC[mode] = _build(mode)
    return _NC[mode]


def _prep_w(weight, mode):
    wt = weight.astype(np.float32, copy=False).T  # [IN, OUT]
    if mode == "fp16x1":
        return [np.ascontiguousarray(wt.astype(np.float16))]
    w_hi = wt.astype(ml_dtypes.bfloat16)
    w_lo = (wt - w_hi.astype(np.float32)).astype(ml_dtypes.bfloat16)
    return [np.ascontiguousarray(w_hi), np.ascontiguousarray(w_lo)]


def kernel(x: np.ndarray, weight: np.ndarray, bias: np.ndarray) -> np.ndarray:
    from concourse.bass_utils import run_bass_kernel_spmd

    global LAST_RESULTS
    nc = _get_nc(MODE)

    x2d = np.ascontiguousarray(x, dtype=np.float32).reshape(B * S, IN)
    ws = _prep_w(weight, MODE)
    bias = np.ascontiguousarray(bias, dtype=np.float32)

    in_maps = []
    for c in range(N_CORES):
        shard = x2d[c * M:(c + 1) * M]
        im = {"xt": np.ascontiguousarray(shard.T), "bias": bias}
        for p, w in enumerate(ws):
            im[f"wt{p}"] = w
        in_maps.append(im)

    LAST_RESULTS = run_bass_kernel_spmd(
        nc, in_maps, core_ids=list(range(N_CORES)))
    out = np.concatenate(
        [LAST_RESULTS.results[c]["y"] for c in range(N_CORES)], axis=0)
    return out.reshape(B, S, OUT).astype(ml_dtypes.bfloat16, copy=False)
